# revision 20
# baseline (speedup 1.0000x reference)
"""Trainium2 Bass kernel for nn_BoxLoss (YOLO-style box regression loss).

Contract: kernel(**inputs) takes FULL unsharded inputs (numpy), returns the
FULL scalar loss. Internally: pure data parallel over batch across 8
NeuronCores (4 images per core); each core computes its 12 (scale, image)
row losses entirely on-device and writes its partial sum; the host adds
the 8 partials while unsharding.

Only ~50 targets x 12 rows of real work exist per core; the big
[B,A,g,g,85] activation tensors are touched ONLY via indirect (gather)
DMAs of the <=600 matched cells x 4 channels the loss actually reads -
the kernel never streams the full tensors.

Layouts:
  matching math   [50, *]  partition = target j, free r = s*4 + b
  gather + loss   [100, *] partition = (b-half, j), free q = s*2 + bl
The indirect-DMA HW consumes ONE index per destination partition, so the
[100,*] layout needs only 6 gathers; the per-scale loss chains run inside
the gather window and the final reduction stays partition-local (PE
matmuls with block-indicator lhsT), so nothing crosses partitions after
the last gather.
"""

import numpy as np

import concourse.bass as bass
import concourse.bacc as bacc
import concourse.mybir as mybir
import concourse.tile as tile

NCORES = 8
GRIDS = (52, 26, 13)
A = 3           # anchors per scale
T = 50          # targets per image
PB = 4          # images per core
R = 3 * PB      # (scale, image) rows per core
BLOCK = 8192.0  # per-row key offset; cells < 3*52*52 = 8112 < 8192
SENT = 8112.0   # sentinel cell id for unmatched targets (>= any real cell)
B_TOTAL = 32

F32 = mybir.dt.float32
I32 = mybir.dt.int32

_SCALE_ELEMS = [PB * A * g * g * 85 for g in GRIDS]
_SCALE_BASE = [0, _SCALE_ELEMS[0], _SCALE_ELEMS[0] + _SCALE_ELEMS[1]]
OUTCAT_ELEMS = sum(_SCALE_ELEMS)

# cstA column layout ([50, _CA_TOT])
_C_G4 = 0         # [0,48)    g per (s,b,c)
_C_JCR = 48       # [48,60)   8112 + r*8192
_C_BGOFF = 60     # [60,72)   scale base + b*3*g^2*85
_C_HW = 72        # [72,84)   g^2
_C_W = 84         # [84,96)   g
_C_EYE = 96       # [96,146)  identity 50x50
_CA_TOT = 146

# cst100 column layout ([100, 10])
_D_ONESU = 0      # [0,2)   block indicator: col u = 1 if p//50 == u
_D_MXY8 = 2       # [2,10)  [1,1,0,0,1,1,0,0]


def _consts():
    r = np.arange(R, dtype=np.float32)[None, :]
    s = (r // PB).astype(np.int64)
    b = (r % PB).astype(np.int64)
    g = np.array(GRIDS, dtype=np.float32)[s]

    g4 = np.broadcast_to(g[:, :, None], (T, R, 4)).reshape(T, 48)
    jcr = np.broadcast_to(SENT + r * BLOCK, (T, R))
    base = np.array(_SCALE_BASE, dtype=np.float64)[s]
    bgoff = np.broadcast_to(base + b * (A * 85) * (g.astype(np.float64) ** 2),
                            (T, R)).astype(np.float32)
    hw4 = np.broadcast_to(g * g, (T, R))
    w4 = np.broadcast_to(g, (T, R))
    eye = np.eye(T, dtype=np.float32)
    cstA = np.concatenate([g4, jcr, bgoff, hw4, w4, eye],
                          axis=1).astype(np.float32)
    assert cstA.shape == (T, _CA_TOT)

    later = np.triu(np.ones((T, T), np.float32), 1)
    cstB = np.ascontiguousarray(
        np.broadcast_to(later[:, None, :], (T, R, T)).reshape(T, R * T))

    onesu = np.zeros((100, 2), np.float32)
    onesu[0:50, 0] = 1.0
    onesu[50:100, 1] = 1.0
    mxy8 = np.broadcast_to(np.array([1, 1, 0, 0], np.float32), (100, 2, 4))
    cst100 = np.concatenate([onesu, mxy8.reshape(100, 8)],
                            axis=1).astype(np.float32)
    return np.ascontiguousarray(cstA), cstB, np.ascontiguousarray(cst100)


def build_nc(use_collective: bool = False):
    nc = bacc.Bacc("TRN2", target_bir_lowering=False, debug=False,
                   num_devices=NCORES)

    tg16_d = nc.dram_tensor("tg16", [T, 16], F32, kind="ExternalInput")
    awh_d = nc.dram_tensor("awh", [1, 72], F32, kind="ExternalInput")
    outcat_d = nc.dram_tensor("outcat", [OUTCAT_ELEMS], F32, kind="ExternalInput")
    loss_d = nc.dram_tensor("loss", [1, 1], F32, kind="ExternalOutput")
    cstA_np, cstB_np, cst100_np = _consts()
    cstA_d = nc.inline_tensor(cstA_np, name="cstA")
    cstB_d = nc.inline_tensor(cstB_np, name="cstB")
    cst100_d = nc.inline_tensor(cst100_np, name="cst100")

    AL = mybir.AluOpType
    AX = mybir.AxisListType.X

    with tile.TileContext(nc) as tc:
        with (
            tc.tile_pool(name="sbuf", bufs=1) as sp,
            tc.tile_pool(name="psum", bufs=1, space="PSUM") as pp,
            tc.tile_pool(name="dram", bufs=1, space="DRAM") as dp,
        ):
            def tt(out, in0, in1, op):
                nc.vector.tensor_tensor(out=out, in0=in0, in1=in1, op=op)

            def ts(out, in0, s1, op, s2=None, op2=None):
                if op2 is None:
                    nc.vector.tensor_scalar(out=out, in0=in0, scalar1=s1,
                                            scalar2=None, op0=op)
                else:
                    nc.vector.tensor_scalar(out=out, in0=in0, scalar1=s1,
                                            scalar2=s2, op0=op, op1=op2)

            def stt(out, in0, scalar, in1, op0, op1):
                nc.vector.scalar_tensor_tensor(out=out, in0=in0, scalar=scalar,
                                               in1=in1, op0=op0, op1=op1)

            _tn = [0]

            def new(shape, dt=F32):
                _tn[0] += 1
                return sp.tile(shape, dt, name=f"t{_tn[0]}")

            # ---------- loads ----------
            tgt = new([T, 16])
            nc.sync.dma_start(out=tgt[:], in_=tg16_d[:, :])
            awhT = new([T, 72])
            nc.sync.dma_start(out=awhT[:], in_=awh_d[:, :].to_broadcast([T, 72]))
            cstA = new([T, _CA_TOT])
            nc.sync.dma_start(out=cstA[:], in_=cstA_d[:, :])
            cstH = new([100, 10])
            nc.scalar.dma_start(out=cstH[:], in_=cst100_d[:, :])
            lat = new([T, R * T])
            nc.gpsimd.dma_start(out=lat[:], in_=cstB_d[:, :])

            def C(c0, w):
                return cstA[:, c0:c0 + w]

            onesU = cstH[:, _D_ONESU:_D_ONESU + 2]
            MXY8 = cstH[:, _D_MXY8:_D_MXY8 + 8]

            ones2 = new([2, 1])
            nc.vector.memset(ones2[:], 1.0)

            # ---------- validity (dep: tgt only) ----------
            sv = new([T, 4])
            nc.vector.reduce_sum(out=sv[:],
                                 in_=tgt[:].rearrange("p (b c) -> p b c", c=4),
                                 axis=AX)
            v4 = new([T, 4]); ts(v4[:], sv[:], 0.0, AL.is_gt)

            # ---------- t = raw * g ----------
            t4 = new([T, 48])
            tt(t4[:], tgt[:, None, :].to_broadcast([T, 3, 16]), C(_C_G4, 48),
               AL.mult)
            t4v = t4[:].rearrange("p (r c) -> p r c", c=4)
            txy = t4v[:, :, 0:2]
            twh = t4v[:, :, 2:4]

            # ---------- floor(xy) ----------
            r1 = new([T, 24])
            ts(r1[:], txy, float(2 ** 23), AL.add)
            r2 = new([T, 24])
            ts(r2[:], r1[:], -float(2 ** 23), AL.add)
            gtm = new([T, 24])
            tt(gtm[:], r2[:], txy, AL.is_gt)
            fxy = new([T, 24])
            tt(fxy[:], r2[:], gtm[:], AL.subtract)
            fv = fxy[:].rearrange("p (r q) -> p r q", q=2)
            cx4 = fv[:, :, 0:1]
            cy4 = fv[:, :, 1:2]

            # ---------- target rect ----------
            zt05 = new([T, 24])
            stt(zt05[:], txy, -0.5, fxy[:], AL.add, AL.subtract)
            lo = new([T, 24])
            stt(lo[:], twh, -0.5, zt05[:], AL.mult, AL.add)
            hi = new([T, 24])
            stt(hi[:], twh, 0.5, zt05[:], AL.mult, AL.add)

            # ---------- anchors + IoU in (xy?, r, a) layout ----------
            awhh = new([T, 72]); ts(awhh[:], awhT[:], 0.5, AL.mult)
            nawhh = new([T, 72]); ts(nawhh[:], awhT[:], -0.5, AL.mult)
            areaa = new([T, 36])
            tt(areaa[:], awhT[:, 0:36], awhT[:, 36:72], AL.mult)

            def bc72(t24):
                return (t24[:].rearrange("p (r q) -> p q r", q=2)[:, :, :, None]
                        .to_broadcast([T, 2, 12, 3]))

            P0 = new([T, 72]); tt(P0[:], bc72(lo), nawhh[:], AL.max)
            P1 = new([T, 72]); tt(P1[:], bc72(hi), awhh[:], AL.min)
            # inter = max(x1-x0,0)*max(y1-y0,0)  (== dx*dy*flag bit-exactly)
            D = new([T, 72]); tt(D[:], P1[:], P0[:], AL.subtract)
            M0 = new([T, 72]); ts(M0[:], D[:], 0.0, AL.max)
            inter = new([T, 36]); tt(inter[:], M0[:, 0:36], M0[:, 36:72], AL.mult)
            dT = new([T, 24]); tt(dT[:], hi[:], lo[:], AL.subtract)
            dv = dT[:].rearrange("p (r q) -> p r q", q=2)
            areat = new([T, 12]); tt(areat[:], dv[:, :, 0:1], dv[:, :, 1:2], AL.mult)
            un1 = new([T, 36])
            tt(un1[:], areat[:, :, None].to_broadcast([T, 12, 3]), areaa[:], AL.add)
            union = new([T, 36]); tt(union[:], un1[:], inter[:], AL.subtract)
            runi = new([T, 36]); nc.vector.reciprocal(out=runi[:], in_=union[:])
            iou = new([T, 36]); tt(iou[:], inter[:], runi[:], AL.mult)

            # ---------- overlap / argmax / cell / gather offsets ----------
            overlap = new([T, 12])
            nc.vector.reduce_max(out=overlap[:],
                                 in_=iou[:].rearrange("p (r a) -> p r a", a=3),
                                 axis=AX)
            iv = iou[:].rearrange("p (r a) -> p r a", a=3)
            eq0 = new([T, 12]); tt(eq0[:], iv[:, :, 0:1], overlap[:], AL.is_equal)
            eq1 = new([T, 12]); tt(eq1[:], iv[:, :, 1:2], overlap[:], AL.is_equal)
            t2 = new([T, 12]); ts(t2[:], eq1[:], 0.0, AL.is_equal, 1.0, AL.add)
            neq0 = new([T, 12]); ts(neq0[:], eq0[:], 0.0, AL.is_equal)
            anc = new([T, 12]); tt(anc[:], neq0[:], t2[:], AL.mult)

            ca = new([T, 12]); tt(ca[:], anc[:], C(_C_HW, 12), AL.mult)
            cb = new([T, 12]); tt(cb[:], cy4, C(_C_W, 12), AL.mult)
            cc = new([T, 12]); tt(cc[:], ca[:], cb[:], AL.add)
            cell = new([T, 12]); tt(cell[:], cc[:], cx4, AL.add)
            idf = new([T, 12])
            stt(idf[:], cell[:], 85.0, C(_C_BGOFF, 12), AL.mult, AL.add)
            idxi = new([T, 12], I32)
            nc.vector.tensor_copy(out=idxi[:], in_=idf[:])

            # indices to [100,6]: partition p = bh*50+j, col q = s*2+bl
            idx2 = new([100, 6], I32)
            iview = idxi[:].rearrange("p (s bh bl) -> p s bh bl", bh=2, bl=2)
            nc.scalar.dma_start(out=idx2[0:50, :], in_=iview[:, :, 0, :])
            nc.sync.dma_start(out=idx2[50:100, :], in_=iview[:, :, 1, :])

            # ---------- dedup (runs while gathers execute) ----------
            om = new([T, 12]); ts(om[:], overlap[:], 0.5, AL.is_gt)
            m = new([T, 12])
            tt(m[:], om[:].rearrange("p (s b) -> p s b", b=4),
               v4[:, None, :].to_broadcast([T, 3, 4]), AL.mult)
            kk = new([T, 12])
            stt(kk[:], cell[:], -SENT, m[:], AL.add, AL.mult)
            key = new([T, 12]); tt(key[:], kk[:], C(_C_JCR, 12), AL.add)

            keyT_p = pp.tile([R, T], F32, name="keyT_p")
            nc.tensor.matmul(out=keyT_p[:], lhsT=key[:], rhs=C(_C_EYE, T),
                             start=True, stop=True)
            keyT = new([R, T])
            nc.vector.tensor_copy(out=keyT[:], in_=keyT_p[:])
            kd2 = nc.dram_tensor("kd2", [R * T], F32)
            nc.sync.dma_start(out=kd2[:].rearrange("(r k) -> r k", k=T),
                              in_=keyT[:])
            keyB = new([T, R * T])
            nc.sync.dma_start(out=keyB[:],
                              in_=kd2[:].unsqueeze(0).to_broadcast([T, R * T]))

            # ---------- t in gather layout + rsqrt(t_wh) (early) ----------
            t42 = new([100, 24])
            tv = t4[:].rearrange("p (s u) -> p s u", u=16)
            nc.scalar.dma_start(out=t42[0:50, :], in_=tv[:, :, 0:8])
            nc.sync.dma_start(out=t42[50:100, :], in_=tv[:, :, 8:16])
            t42v = t42[:].rearrange("p (s bl c) -> p s bl c", bl=2, c=4)
            rwh2 = new([100, 12])
            nc.vector.reciprocal(out=rwh2[:], in_=t42v[:, :, :, 2:4])
            rstw = new([100, 12]); nc.scalar.sqrt(out=rstw[:], in_=rwh2[:])

            # ---------- 6 indirect gathers (3 pair tiles) + loss chains ----
            gpair = [new([100, 8]) for _ in range(3)]
            for q in range(6):
                s_, bl = q // 2, q % 2
                nc.gpsimd.indirect_dma_start(
                    out=gpair[s_][:, bl * 4:(bl + 1) * 4], out_offset=None,
                    in_=outcat_d[:].unsqueeze(1),
                    in_offset=bass.IndirectOffsetOnAxis(ap=idx2[:, q:q + 1],
                                                        axis=0),
                )

            TS2 = new([100, 6])
            winner = new([T, 12])
            winner2 = new([100, 12])   # cols 0:6 winner, cols 6:12 winner*TS

            def stripe_chain(s_):
                g8 = gpair[s_]
                gv = g8[:].rearrange("p (bl c) -> p bl c", c=4)
                t8 = t42v[:, s_, :, :]
                rcpw = new([100, 4])
                nc.vector.reciprocal(out=rcpw[:], in_=gv[:, :, 2:4])
                rspw = new([100, 4]); nc.scalar.sqrt(out=rspw[:], in_=rcpw[:])
                sel = new([100, 8])
                selv = sel[:].rearrange("p (bl c) -> p bl c", c=4)
                tt(selv[:, :, 0:2], gv[:, :, 0:2], t8[:, :, 0:2], AL.subtract)
                tt(selv[:, :, 2:4], rspw[:], rstw[:, 4 * s_:4 * s_ + 4],
                   AL.subtract)
                sq = new([100, 8]); tt(sq[:], sel[:], sel[:], AL.mult)
                nc.vector.reduce_sum(
                    out=TS2[:, 2 * s_:2 * s_ + 2],
                    in_=sq[:].rearrange("p (bl c) -> p bl c", c=4), axis=AX)

            # scale 0 chain (ready first)
            stripe_chain(0)

            # dedup tail -> winner, relocation, n / rden (gather window)
            E = new([T, R * T])
            tt(E[:], key[:, :, None].to_broadcast([T, 12, T]), keyB[:],
               AL.is_equal)
            EL = new([T, R * T])
            tt(EL[:], E[:], lat[:], AL.mult)
            ov = new([T, 12])
            nc.vector.reduce_max(out=ov[:],
                                 in_=EL[:].rearrange("p (r k) -> p r k", k=T),
                                 axis=AX)
            nov = new([T, 12]); ts(nov[:], ov[:], 0.0, AL.is_equal)
            tt(winner[:], m[:], nov[:], AL.mult)
            wv = winner[:].rearrange("p (s bh bl) -> p s bh bl", bh=2, bl=2)
            nc.scalar.dma_start(out=winner2[0:50, 0:6], in_=wv[:, :, 0, :])
            nc.sync.dma_start(out=winner2[50:100, 0:6], in_=wv[:, :, 1, :])

            stripe_chain(1)
            stripe_chain(2)

            # ---------- partition-local final reduction ----------
            tt(winner2[:, 6:12], TS2[:], winner2[:, 0:6], AL.mult)
            M1_p = pp.tile([2, 12], F32, name="M1_p")
            nc.tensor.matmul(out=M1_p[:], lhsT=onesU, rhs=winner2[:],
                             start=True, stop=True)
            mx2 = new([2, 6])
            ts(mx2[:], M1_p[:, 0:6], 1.0, AL.max, 2.0, AL.mult)
            rden2 = new([2, 6]); nc.vector.reciprocal(out=rden2[:], in_=mx2[:])
            rl2 = new([2, 6]); tt(rl2[:], M1_p[:, 6:12], rden2[:], AL.mult)
            pt2 = new([2, 1])
            nc.vector.reduce_sum(out=pt2[:], in_=rl2[:], axis=AX)
            tot_p = pp.tile([1, 1], F32, name="tot_p")
            nc.tensor.matmul(out=tot_p[:], lhsT=ones2[:], rhs=pt2[:],
                             start=True, stop=True)
            p32 = new([1, 1])
            ts(p32[:], tot_p[:], 1.0 / B_TOTAL, AL.mult)

            if use_collective:
                ccin = dp.tile([1, 1], F32, name="ccin")
                ccout = dp.tile([1, 1], F32, name="ccout")
                nc.sync.dma_start(out=ccin[:], in_=p32[:])
                nc.gpsimd.collective_compute(
                    "AllReduce", AL.add,
                    replica_groups=[list(range(NCORES))],
                    ins=[ccin[:].opt()], outs=[ccout[:].opt()],
                )
                nc.sync.dma_start(out=loss_d[:, :], in_=ccout[:])
            else:
                nc.sync.dma_start(out=loss_d[:, :], in_=p32[:])

    nc.compile()
    return nc


def make_in_maps(output0, anchors0, output1, anchors1, output2, anchors2,
                 targets):
    outs = [np.asarray(output0), np.asarray(output1), np.asarray(output2)]
    ancs = [np.asarray(anchors0), np.asarray(anchors1), np.asarray(anchors2)]
    tg = np.asarray(targets)

    aw = np.concatenate([np.tile(a[:, 0], PB) for a in ancs])  # [36] (r, a)
    ah = np.concatenate([np.tile(a[:, 1], PB) for a in ancs])
    awh = np.concatenate([aw, ah]).astype(np.float32)[None, :]  # [1,72]

    in_maps = []
    for c in range(NCORES):
        sl = slice(c * PB, (c + 1) * PB)
        tg16 = np.ascontiguousarray(
            tg[sl, :, 1:5].transpose(1, 0, 2).reshape(T, 16).astype(np.float32))
        outcat = np.concatenate([o[sl].ravel() for o in outs]).astype(np.float32)
        in_maps.append({"tg16": tg16, "awh": awh, "outcat": outcat})
    return in_maps


_NC_CACHE = {}


def kernel(output0, anchors0, output1, anchors1, output2, anchors2, targets):
    from concourse.bass_utils import run_bass_kernel_spmd

    if "nc" not in _NC_CACHE:
        _NC_CACHE["nc"] = build_nc(use_collective=False)
    nc = _NC_CACHE["nc"]
    in_maps = make_in_maps(output0, anchors0, output1, anchors1, output2,
                           anchors2, targets)
    res = run_bass_kernel_spmd(nc, in_maps, list(range(NCORES)))
    total = np.float32(0.0)
    for c in range(NCORES):
        total += np.float32(res.results[c]["loss"].reshape(()))
    return np.float32(total)


# revision 21
# speedup vs baseline: 1.0248x; 1.0248x over previous
"""Trainium2 Bass kernel for nn_BoxLoss (YOLO-style box regression loss).

Contract: kernel(**inputs) takes FULL unsharded inputs (numpy), returns the
FULL scalar loss. Internally: pure data parallel over batch across 8
NeuronCores (4 images per core); each core computes its 12 (scale, image)
row losses entirely on-device and writes its partial sum; the host adds
the 8 partials while unsharding.

Only ~50 targets x 12 rows of real work exist per core; the big
[B,A,g,g,85] activation tensors are touched ONLY via indirect (gather)
DMAs of the <=600 matched cells x 4 channels the loss actually reads -
the kernel never streams the full tensors.

Layouts:
  matching math   [50, *]  partition = target j, free r = s*4 + b
  gather + loss   [100, *] partition = (b-half, j), free q = s*2 + bl
The indirect-DMA HW consumes ONE index per destination partition, so the
[100,*] layout needs only 6 gathers; the per-scale loss chains run inside
the gather window and the final reduction stays partition-local (PE
matmuls with block-indicator lhsT). HWDGE queue entries are emitted in
expected-readiness order (FIFO head-of-line blocking otherwise stalls
later-emitted but earlier-ready transfers).
"""

import numpy as np

import concourse.bass as bass
import concourse.bacc as bacc
import concourse.mybir as mybir
import concourse.tile as tile

NCORES = 8
GRIDS = (52, 26, 13)
A = 3           # anchors per scale
T = 50          # targets per image
PB = 4          # images per core
R = 3 * PB      # (scale, image) rows per core
BLOCK = 8192.0  # per-row key offset; cells < 3*52*52 = 8112 < 8192
SENT = 8112.0   # sentinel cell id for unmatched targets (>= any real cell)
B_TOTAL = 32

F32 = mybir.dt.float32
I32 = mybir.dt.int32

_SCALE_ELEMS = [PB * A * g * g * 85 for g in GRIDS]
_SCALE_BASE = [0, _SCALE_ELEMS[0], _SCALE_ELEMS[0] + _SCALE_ELEMS[1]]
OUTCAT_ELEMS = sum(_SCALE_ELEMS)

# hostpack column layout ([50, _HP_TOT]): runtime data + replicated consts
_H_TGT = 0        # [0,16)    targets (j; b, ch) slice
_H_AWH = 16       # [16,88)   anchor w/h replicated (r, a)
_H_G4 = 88        # [88,136)  g per (s,b,c)
_H_JCR = 136      # [136,148) 8112 + r*8192
_H_BGOFF = 148    # [148,160) scale base + b*3*g^2*85
_H_HW = 160       # [160,172) g^2
_H_W = 172        # [172,184) g
_H_EYE = 184      # [184,234) identity 50x50
_HP_TOT = 234

# cst100 column layout ([100, 11])
_D_ONESU = 0      # [0,2)   block indicator: col u = 1 if p//50 == u
_D_ONE = 2        # [2,3)   1.0
_D_PAD = 3


def _host_consts():
    """The [50, 146] constant tail of hostpack (grid structure only)."""
    r = np.arange(R, dtype=np.float32)[None, :]
    s = (r // PB).astype(np.int64)
    b = (r % PB).astype(np.int64)
    g = np.array(GRIDS, dtype=np.float32)[s]

    g4 = np.broadcast_to(g[:, :, None], (T, R, 4)).reshape(T, 48)
    jcr = np.broadcast_to(SENT + r * BLOCK, (T, R))
    base = np.array(_SCALE_BASE, dtype=np.float64)[s]
    bgoff = np.broadcast_to(base + b * (A * 85) * (g.astype(np.float64) ** 2),
                            (T, R)).astype(np.float32)
    hw4 = np.broadcast_to(g * g, (T, R))
    w4 = np.broadcast_to(g, (T, R))
    eye = np.eye(T, dtype=np.float32)
    return np.concatenate([g4, jcr, bgoff, hw4, w4, eye],
                          axis=1).astype(np.float32)


def _inline_consts():
    later = np.triu(np.ones((T, T), np.float32), 1)
    cstB = np.ascontiguousarray(
        np.broadcast_to(later[:, None, :], (T, R, T)).reshape(T, R * T))

    cst100 = np.zeros((100, _D_PAD), np.float32)
    cst100[0:50, _D_ONESU] = 1.0
    cst100[50:100, _D_ONESU + 1] = 1.0
    cst100[:, _D_ONE] = 1.0
    return cstB, np.ascontiguousarray(cst100)


def build_nc(use_collective: bool = False):
    nc = bacc.Bacc("TRN2", target_bir_lowering=False, debug=False,
                   num_devices=NCORES)

    hp_d = nc.dram_tensor("hostpack", [T, _HP_TOT], F32, kind="ExternalInput")
    outcat_d = nc.dram_tensor("outcat", [OUTCAT_ELEMS], F32, kind="ExternalInput")
    loss_d = nc.dram_tensor("loss", [1, 1], F32, kind="ExternalOutput")
    cstB_np, cst100_np = _inline_consts()
    cstB_d = nc.inline_tensor(cstB_np, name="cstB")
    cst100_d = nc.inline_tensor(cst100_np, name="cst100")

    AL = mybir.AluOpType
    AX = mybir.AxisListType.X

    with tile.TileContext(nc) as tc:
        with (
            tc.tile_pool(name="sbuf", bufs=1) as sp,
            tc.tile_pool(name="psum", bufs=1, space="PSUM") as pp,
            tc.tile_pool(name="dram", bufs=1, space="DRAM") as dp,
        ):
            def tt(out, in0, in1, op):
                nc.vector.tensor_tensor(out=out, in0=in0, in1=in1, op=op)

            def ts(out, in0, s1, op, s2=None, op2=None):
                if op2 is None:
                    nc.vector.tensor_scalar(out=out, in0=in0, scalar1=s1,
                                            scalar2=None, op0=op)
                else:
                    nc.vector.tensor_scalar(out=out, in0=in0, scalar1=s1,
                                            scalar2=s2, op0=op, op1=op2)

            def stt(out, in0, scalar, in1, op0, op1):
                nc.vector.scalar_tensor_tensor(out=out, in0=in0, scalar=scalar,
                                               in1=in1, op0=op0, op1=op1)

            _tn = [0]

            def new(shape, dt=F32):
                _tn[0] += 1
                return sp.tile(shape, dt, name=f"t{_tn[0]}")

            # ---------- loads (readiness-ordered per HWDGE ring) ----------
            # ACT ring: lat(120KB, no deps) first; then cstH; later t42a/
            # idx2a/winner2a in readiness order.
            lat = new([T, R * T])
            nc.scalar.dma_start(out=lat[:], in_=cstB_d[:, :])
            cstH = new([100, _D_PAD])
            nc.scalar.dma_start(out=cstH[:], in_=cst100_d[:, :])
            # sync ring: hostpack first.
            hp = new([T, _HP_TOT])
            nc.sync.dma_start(out=hp[:], in_=hp_d[:, :])

            def C(c0, w):
                return hp[:, c0:c0 + w]

            tgt = C(_H_TGT, 16)
            awhT = C(_H_AWH, 72)
            onesU = cstH[:, _D_ONESU:_D_ONESU + 2]
            ones2 = cstH[0:2, _D_ONE:_D_ONE + 1]

            # ---------- validity ----------
            sv = new([T, 4])
            nc.vector.reduce_sum(out=sv[:],
                                 in_=tgt.rearrange("p (b c) -> p b c", c=4),
                                 axis=AX)
            v4 = new([T, 4]); ts(v4[:], sv[:], 0.0, AL.is_gt)

            # ---------- t = raw * g ----------
            t4 = new([T, 48])
            tt(t4[:], tgt[:, None, :].to_broadcast([T, 3, 16]), C(_H_G4, 48),
               AL.mult)
            t4v = t4[:].rearrange("p (r c) -> p r c", c=4)
            txy = t4v[:, :, 0:2]
            twh = t4v[:, :, 2:4]

            # t in gather layout (dep: t4 only -> early ring slots)
            t42 = new([100, 24])
            tv = t4[:].rearrange("p (s u) -> p s u", u=16)
            nc.scalar.dma_start(out=t42[0:50, :], in_=tv[:, :, 0:8])
            nc.sync.dma_start(out=t42[50:100, :], in_=tv[:, :, 8:16])

            # ---------- floor(xy) ----------
            r1 = new([T, 24])
            ts(r1[:], txy, float(2 ** 23), AL.add)
            r2 = new([T, 24])
            ts(r2[:], r1[:], -float(2 ** 23), AL.add)
            gtm = new([T, 24])
            tt(gtm[:], r2[:], txy, AL.is_gt)
            fxy = new([T, 24])
            tt(fxy[:], r2[:], gtm[:], AL.subtract)
            fv = fxy[:].rearrange("p (r q) -> p r q", q=2)
            cx4 = fv[:, :, 0:1]
            cy4 = fv[:, :, 1:2]

            # ---------- target rect ----------
            zt05 = new([T, 24])
            stt(zt05[:], txy, -0.5, fxy[:], AL.add, AL.subtract)
            lo = new([T, 24])
            stt(lo[:], twh, -0.5, zt05[:], AL.mult, AL.add)
            hi = new([T, 24])
            stt(hi[:], twh, 0.5, zt05[:], AL.mult, AL.add)

            # ---------- anchors + IoU in (q, r, a) layout ----------
            awhh = new([T, 72]); ts(awhh[:], awhT, 0.5, AL.mult)
            nawhh = new([T, 72]); ts(nawhh[:], awhT, -0.5, AL.mult)
            areaa = new([T, 36])
            tt(areaa[:], awhT[:, 0:36], awhT[:, 36:72], AL.mult)

            def bc72(t24):
                return (t24[:].rearrange("p (r q) -> p q r", q=2)[:, :, :, None]
                        .to_broadcast([T, 2, 12, 3]))

            P0 = new([T, 72]); tt(P0[:], bc72(lo), nawhh[:], AL.max)
            P1 = new([T, 72]); tt(P1[:], bc72(hi), awhh[:], AL.min)
            D = new([T, 72]); tt(D[:], P1[:], P0[:], AL.subtract)
            M0 = new([T, 72]); ts(M0[:], D[:], 0.0, AL.max)
            inter = new([T, 36]); tt(inter[:], M0[:, 0:36], M0[:, 36:72], AL.mult)
            dT = new([T, 24]); tt(dT[:], hi[:], lo[:], AL.subtract)
            dv = dT[:].rearrange("p (r q) -> p r q", q=2)
            areat = new([T, 12]); tt(areat[:], dv[:, :, 0:1], dv[:, :, 1:2], AL.mult)
            un1 = new([T, 36])
            tt(un1[:], areat[:, :, None].to_broadcast([T, 12, 3]), areaa[:], AL.add)
            union = new([T, 36]); tt(union[:], un1[:], inter[:], AL.subtract)
            runi = new([T, 36]); nc.vector.reciprocal(out=runi[:], in_=union[:])
            iou = new([T, 36]); tt(iou[:], inter[:], runi[:], AL.mult)

            # ---------- overlap / argmax / cell / gather offsets ----------
            overlap = new([T, 12])
            nc.vector.reduce_max(out=overlap[:],
                                 in_=iou[:].rearrange("p (r a) -> p r a", a=3),
                                 axis=AX)
            iv = iou[:].rearrange("p (r a) -> p r a", a=3)
            eqB = new([T, 24])
            tt(eqB[:], iv[:, :, 0:2],
               overlap[:, :, None].to_broadcast([T, 12, 2]), AL.is_equal)
            ev = eqB[:].rearrange("p (r e) -> p r e", e=2)
            t2 = new([T, 12]); ts(t2[:], ev[:, :, 1:2], 0.0, AL.is_equal, 1.0, AL.add)
            neq0 = new([T, 12]); ts(neq0[:], ev[:, :, 0:1], 0.0, AL.is_equal)
            anc = new([T, 12]); tt(anc[:], neq0[:], t2[:], AL.mult)

            ca = new([T, 12]); tt(ca[:], anc[:], C(_H_HW, 12), AL.mult)
            cb = new([T, 12]); tt(cb[:], cy4, C(_H_W, 12), AL.mult)
            cc = new([T, 12]); tt(cc[:], ca[:], cb[:], AL.add)
            cell = new([T, 12]); tt(cell[:], cc[:], cx4, AL.add)
            idf = new([T, 12])
            stt(idf[:], cell[:], 85.0, C(_H_BGOFF, 12), AL.mult, AL.add)
            idxi = new([T, 12], I32)
            nc.vector.tensor_copy(out=idxi[:], in_=idf[:])

            # indices to [100,6]: partition p = bh*50+j, col q = s*2+bl
            idx2 = new([100, 6], I32)
            iview = idxi[:].rearrange("p (s bh bl) -> p s bh bl", bh=2, bl=2)
            nc.scalar.dma_start(out=idx2[0:50, :], in_=iview[:, :, 0, :])
            nc.sync.dma_start(out=idx2[50:100, :], in_=iview[:, :, 1, :])

            # ---------- dedup key -> winner (overlaps the gathers) ----------
            om = new([T, 12]); ts(om[:], overlap[:], 0.5, AL.is_gt)
            m = new([T, 12])
            tt(m[:], om[:].rearrange("p (s b) -> p s b", b=4),
               v4[:, None, :].to_broadcast([T, 3, 4]), AL.mult)
            kk = new([T, 12])
            stt(kk[:], cell[:], -SENT, m[:], AL.add, AL.mult)
            key = new([T, 12]); tt(key[:], kk[:], C(_H_JCR, 12), AL.add)

            keyT_p = pp.tile([R, T], F32, name="keyT_p")
            nc.tensor.matmul(out=keyT_p[:], lhsT=key[:], rhs=C(_H_EYE, T),
                             start=True, stop=True)
            keyT = new([R, T])
            nc.vector.tensor_copy(out=keyT[:], in_=keyT_p[:])
            kd2 = nc.dram_tensor("kd2", [R * T], F32)
            nc.sync.dma_start(out=kd2[:].rearrange("(r k) -> r k", k=T),
                              in_=keyT[:])
            keyB = new([T, R * T])
            nc.sync.dma_start(out=keyB[:],
                              in_=kd2[:].unsqueeze(0).to_broadcast([T, R * T]))

            # rsqrt of t_wh (dep: t42)
            t42v = t42[:].rearrange("p (s bl c) -> p s bl c", bl=2, c=4)
            rwh2 = new([100, 12])
            nc.vector.reciprocal(out=rwh2[:], in_=t42v[:, :, :, 2:4])
            rstw = new([100, 12]); nc.scalar.sqrt(out=rstw[:], in_=rwh2[:])

            # ---------- 6 indirect gathers (3 pair tiles) ----------
            gpair = [new([100, 8]) for _ in range(3)]
            for q in range(6):
                s_, bl = q // 2, q % 2
                nc.gpsimd.indirect_dma_start(
                    out=gpair[s_][:, bl * 4:(bl + 1) * 4], out_offset=None,
                    in_=outcat_d[:].unsqueeze(1),
                    in_offset=bass.IndirectOffsetOnAxis(ap=idx2[:, q:q + 1],
                                                        axis=0),
                )

            TS2 = new([100, 6])
            winner = new([T, 12])
            winner2 = new([100, 12])   # cols 0:6 winner, cols 6:12 winner*TS

            # dedup tail first on DVE (keyB lands before gather pair 0)
            E = new([T, R * T])
            tt(E[:], key[:, :, None].to_broadcast([T, 12, T]), keyB[:],
               AL.is_equal)
            EL = new([T, R * T])
            tt(EL[:], E[:], lat[:], AL.mult)
            ov = new([T, 12])
            nc.vector.reduce_max(out=ov[:],
                                 in_=EL[:].rearrange("p (r k) -> p r k", k=T),
                                 axis=AX)
            nov = new([T, 12]); ts(nov[:], ov[:], 0.0, AL.is_equal)
            tt(winner[:], m[:], nov[:], AL.mult)
            wv = winner[:].rearrange("p (s bh bl) -> p s bh bl", bh=2, bl=2)
            nc.scalar.dma_start(out=winner2[0:50, 0:6], in_=wv[:, :, 0, :])
            nc.sync.dma_start(out=winner2[50:100, 0:6], in_=wv[:, :, 1, :])

            def stripe_chain(s_):
                g8 = gpair[s_]
                gv = g8[:].rearrange("p (bl c) -> p bl c", c=4)
                t8 = t42v[:, s_, :, :]
                rcpw = new([100, 4])
                nc.vector.reciprocal(out=rcpw[:], in_=gv[:, :, 2:4])
                rspw = new([100, 4]); nc.scalar.sqrt(out=rspw[:], in_=rcpw[:])
                sel = new([100, 8])
                selv = sel[:].rearrange("p (bl c) -> p bl c", c=4)
                tt(selv[:, :, 0:2], gv[:, :, 0:2], t8[:, :, 0:2], AL.subtract)
                tt(selv[:, :, 2:4], rspw[:], rstw[:, 4 * s_:4 * s_ + 4],
                   AL.subtract)
                sq = new([100, 8]); tt(sq[:], sel[:], sel[:], AL.mult)
                nc.vector.reduce_sum(
                    out=TS2[:, 2 * s_:2 * s_ + 2],
                    in_=sq[:].rearrange("p (bl c) -> p bl c", c=4), axis=AX)

            stripe_chain(0)
            stripe_chain(1)
            stripe_chain(2)

            # ---------- partition-local final reduction ----------
            tt(winner2[:, 6:12], TS2[:], winner2[:, 0:6], AL.mult)
            M1_p = pp.tile([2, 12], F32, name="M1_p")
            nc.tensor.matmul(out=M1_p[:], lhsT=onesU, rhs=winner2[:],
                             start=True, stop=True)
            mx2 = new([2, 6])
            ts(mx2[:], M1_p[:, 0:6], 1.0, AL.max, 2.0, AL.mult)
            rden2 = new([2, 6]); nc.vector.reciprocal(out=rden2[:], in_=mx2[:])
            rl2 = new([2, 6]); tt(rl2[:], M1_p[:, 6:12], rden2[:], AL.mult)
            pt2 = new([2, 1])
            nc.vector.reduce_sum(out=pt2[:], in_=rl2[:], axis=AX)
            tot_p = pp.tile([1, 1], F32, name="tot_p")
            nc.tensor.matmul(out=tot_p[:], lhsT=ones2, rhs=pt2[:],
                             start=True, stop=True)
            p32 = new([1, 1])
            ts(p32[:], tot_p[:], 1.0 / B_TOTAL, AL.mult)

            if use_collective:
                ccin = dp.tile([1, 1], F32, name="ccin")
                ccout = dp.tile([1, 1], F32, name="ccout")
                nc.sync.dma_start(out=ccin[:], in_=p32[:])
                nc.gpsimd.collective_compute(
                    "AllReduce", AL.add,
                    replica_groups=[list(range(NCORES))],
                    ins=[ccin[:].opt()], outs=[ccout[:].opt()],
                )
                nc.sync.dma_start(out=loss_d[:, :], in_=ccout[:])
            else:
                nc.sync.dma_start(out=loss_d[:, :], in_=p32[:])

    nc.compile()
    return nc


_HOST_CONSTS = _host_consts()


def make_in_maps(output0, anchors0, output1, anchors1, output2, anchors2,
                 targets):
    outs = [np.asarray(output0), np.asarray(output1), np.asarray(output2)]
    ancs = [np.asarray(anchors0), np.asarray(anchors1), np.asarray(anchors2)]
    tg = np.asarray(targets)

    aw = np.concatenate([np.tile(a[:, 0], PB) for a in ancs])  # [36] (r, a)
    ah = np.concatenate([np.tile(a[:, 1], PB) for a in ancs])
    awh = np.broadcast_to(np.concatenate([aw, ah]).astype(np.float32), (T, 72))

    in_maps = []
    for c in range(NCORES):
        sl = slice(c * PB, (c + 1) * PB)
        tg16 = tg[sl, :, 1:5].transpose(1, 0, 2).reshape(T, 16).astype(np.float32)
        hostpack = np.ascontiguousarray(
            np.concatenate([tg16, awh, _HOST_CONSTS], axis=1))
        outcat = np.concatenate([o[sl].ravel() for o in outs]).astype(np.float32)
        in_maps.append({"hostpack": hostpack, "outcat": outcat})
    return in_maps


_NC_CACHE = {}


def kernel(output0, anchors0, output1, anchors1, output2, anchors2, targets):
    from concourse.bass_utils import run_bass_kernel_spmd

    if "nc" not in _NC_CACHE:
        _NC_CACHE["nc"] = build_nc(use_collective=False)
    nc = _NC_CACHE["nc"]
    in_maps = make_in_maps(output0, anchors0, output1, anchors1, output2,
                           anchors2, targets)
    res = run_bass_kernel_spmd(nc, in_maps, list(range(NCORES)))
    total = np.float32(0.0)
    for c in range(NCORES):
        total += np.float32(res.results[c]["loss"].reshape(()))
    return np.float32(total)


# revision 30
# speedup vs baseline: 1.0575x; 1.0320x over previous
"""Trainium2 Bass kernel for nn_BoxLoss (YOLO-style box regression loss).

Contract: kernel(**inputs) takes FULL unsharded inputs (numpy), returns the
FULL scalar loss. Internally: pure data parallel over batch across 8
NeuronCores (4 images per core); each core computes its 12 (scale, image)
row losses entirely on-device and writes its partial sum; the host adds
the 8 partials while unsharding.

Only ~50 targets x 12 rows of real work exist per core; the big
[B,A,g,g,85] activation tensors are touched ONLY via indirect (gather)
DMAs of the <=600 matched cells x 4 channels the loss actually reads -
the kernel never streams the full tensors.

Layouts:
  matching math   [50, *]  partition = target j, free r = s*4 + b
  gather + loss   [100, *] partition = (b-half, j), free q = s*2 + bl
The indirect-DMA HW consumes ONE index per destination partition, so the
[100,*] layout needs only 6 gathers; the per-scale loss chains run inside
the gather window and the final reduction stays partition-local (PE
matmuls with block-indicator lhsT). HWDGE queue entries are emitted in
expected-readiness order (FIFO head-of-line blocking otherwise stalls
later-emitted but earlier-ready transfers).
"""

import numpy as np

import concourse.bass as bass
import concourse.bacc as bacc
import concourse.mybir as mybir
import concourse.tile as tile

NCORES = 8
GRIDS = (52, 26, 13)
A = 3           # anchors per scale
T = 50          # targets per image
PB = 4          # images per core
R = 3 * PB      # (scale, image) rows per core
BLOCK = 8192.0  # per-row key offset; cells < 3*52*52 = 8112 < 8192
SENT = 8112.0   # sentinel cell id for unmatched targets (>= any real cell)
B_TOTAL = 32

F32 = mybir.dt.float32
I32 = mybir.dt.int32

_SCALE_ELEMS = [PB * A * g * g * 85 for g in GRIDS]
_SCALE_BASE = [0, _SCALE_ELEMS[0], _SCALE_ELEMS[0] + _SCALE_ELEMS[1]]
OUTCAT_ELEMS = sum(_SCALE_ELEMS)

# hostpack column layout ([50, _HP_TOT]): runtime data + replicated consts
_H_TGT = 0        # [0,16)    targets (j; b, ch) slice
_H_AWH = 16       # [16,88)   anchor w/h replicated (r, a)
_H_G4 = 88        # [88,136)  g per (s,b,c)
_H_BGOFF = 136    # [136,148) scale base + b*3*g^2*85
_H_HW = 148       # [148,160) g^2
_H_W = 160        # [160,172) g
_H_EYE = 172      # [172,222) identity 50x50
_HP_TOT = 222

# cst100 column layout ([100, 11])
_D_ONESU = 0      # [0,2)   block indicator: col u = 1 if p//50 == u
_D_ONE = 2        # [2,3)   1.0
_D_PAD = 3


def _host_consts():
    """The [50, 146] constant tail of hostpack (grid structure only)."""
    r = np.arange(R, dtype=np.float32)[None, :]
    s = (r // PB).astype(np.int64)
    b = (r % PB).astype(np.int64)
    g = np.array(GRIDS, dtype=np.float32)[s]

    g4 = np.broadcast_to(g[:, :, None], (T, R, 4)).reshape(T, 48)
    base = np.array(_SCALE_BASE, dtype=np.float64)[s]
    bgoff = np.broadcast_to(base + b * (A * 85) * (g.astype(np.float64) ** 2),
                            (T, R)).astype(np.float32)
    hw4 = np.broadcast_to(g * g, (T, R))
    w4 = np.broadcast_to(g, (T, R))
    eye = np.eye(T, dtype=np.float32)
    return np.concatenate([g4, bgoff, hw4, w4, eye],
                          axis=1).astype(np.float32)


def _inline_consts():
    later = np.triu(np.ones((T, T), np.int16), 1)
    cstB = np.ascontiguousarray(
        np.broadcast_to(later[:, None, :], (T, R, T)).reshape(T, R * T))

    cst100 = np.zeros((100, _D_PAD), np.float32)
    cst100[0:50, _D_ONESU] = 1.0
    cst100[50:100, _D_ONESU + 1] = 1.0
    cst100[:, _D_ONE] = 1.0
    return cstB, np.ascontiguousarray(cst100)


def build_nc(use_collective: bool = False):
    nc = bacc.Bacc("TRN2", target_bir_lowering=False, debug=False,
                   num_devices=NCORES)

    hp_d = nc.dram_tensor("hostpack", [T, _HP_TOT], F32, kind="ExternalInput")
    outcat_d = nc.dram_tensor("outcat", [OUTCAT_ELEMS], F32, kind="ExternalInput")
    loss_d = nc.dram_tensor("loss", [1, 1], F32, kind="ExternalOutput")
    cstB_np, cst100_np = _inline_consts()
    cstB_d = nc.inline_tensor(cstB_np, name="cstB")
    cst100_d = nc.inline_tensor(cst100_np, name="cst100")

    AL = mybir.AluOpType
    AX = mybir.AxisListType.X

    with tile.TileContext(nc) as tc:
        with (
            tc.tile_pool(name="sbuf", bufs=1) as sp,
            tc.tile_pool(name="psum", bufs=1, space="PSUM") as pp,
            tc.tile_pool(name="dram", bufs=1, space="DRAM") as dp,
        ):
            def tt(out, in0, in1, op):
                return nc.vector.tensor_tensor(out=out, in0=in0, in1=in1, op=op)

            def ts(out, in0, s1, op, s2=None, op2=None):
                if op2 is None:
                    nc.vector.tensor_scalar(out=out, in0=in0, scalar1=s1,
                                            scalar2=None, op0=op)
                else:
                    nc.vector.tensor_scalar(out=out, in0=in0, scalar1=s1,
                                            scalar2=s2, op0=op, op1=op2)

            def stt(out, in0, scalar, in1, op0, op1):
                nc.vector.scalar_tensor_tensor(out=out, in0=in0, scalar=scalar,
                                               in1=in1, op0=op0, op1=op1)

            _tn = [0]

            def new(shape, dt=F32):
                _tn[0] += 1
                return sp.tile(shape, dt, name=f"t{_tn[0]}")

            # ---------- loads (readiness-ordered per HWDGE ring) ----------
            # ACT ring: lat(120KB, no deps) first; then cstH; later t42a/
            # idx2a/winner2a in readiness order.
            lat = new([T, R * T], mybir.dt.int16)
            nc.scalar.dma_start(out=lat[:], in_=cstB_d[:, :])
            cstH = new([100, _D_PAD])
            nc.scalar.dma_start(out=cstH[:], in_=cst100_d[:, :])
            # sync ring: hostpack first.
            hp = new([T, _HP_TOT])
            nc.sync.dma_start(out=hp[:], in_=hp_d[:, :])

            def C(c0, w):
                return hp[:, c0:c0 + w]

            tgt = C(_H_TGT, 16)
            awhT = C(_H_AWH, 72)
            onesU = cstH[:, _D_ONESU:_D_ONESU + 2]
            ones2 = cstH[0:2, _D_ONE:_D_ONE + 1]

            # ---------- validity ----------
            sv = new([T, 4])
            nc.vector.reduce_sum(out=sv[:],
                                 in_=tgt.rearrange("p (b c) -> p b c", c=4),
                                 axis=AX)
            v4 = new([T, 4]); ts(v4[:], sv[:], 0.0, AL.is_gt)

            # ---------- t = raw * g ----------
            t4 = new([T, 48])
            tt(t4[:], tgt[:, None, :].to_broadcast([T, 3, 16]), C(_H_G4, 48),
               AL.mult)
            t4v = t4[:].rearrange("p (r c) -> p r c", c=4)
            txy = t4v[:, :, 0:2]
            twh = t4v[:, :, 2:4]

            # t in gather layout (dep: t4 only -> early ring slots)
            t42 = new([100, 24])
            tv = t4[:].rearrange("p (s u) -> p s u", u=16)
            nc.scalar.dma_start(out=t42[0:50, :], in_=tv[:, :, 0:8])
            nc.sync.dma_start(out=t42[50:100, :], in_=tv[:, :, 8:16])

            # ---------- floor(xy) ----------
            r1 = new([T, 24])
            ts(r1[:], txy, float(2 ** 23), AL.add)
            r2 = new([T, 24])
            ts(r2[:], r1[:], -float(2 ** 23), AL.add)
            gtm = new([T, 24])
            tt(gtm[:], r2[:], txy, AL.is_gt)
            fxy = new([T, 24])
            tt(fxy[:], r2[:], gtm[:], AL.subtract)
            fv = fxy[:].rearrange("p (r q) -> p r q", q=2)
            cx4 = fv[:, :, 0:1]
            cy4 = fv[:, :, 1:2]

            # ---------- target rect ----------
            zt05 = new([T, 24])
            stt(zt05[:], txy, -0.5, fxy[:], AL.add, AL.subtract)
            lo = new([T, 24])
            stt(lo[:], twh, -0.5, zt05[:], AL.mult, AL.add)
            hi = new([T, 24])
            stt(hi[:], twh, 0.5, zt05[:], AL.mult, AL.add)

            # ---------- anchors + IoU in (q, r, a) layout ----------
            awhh = new([T, 72]); ts(awhh[:], awhT, 0.5, AL.mult)
            nawhh = new([T, 72]); ts(nawhh[:], awhT, -0.5, AL.mult)
            areaa = new([T, 36])
            tt(areaa[:], awhT[:, 0:36], awhT[:, 36:72], AL.mult)

            def bc72(t24):
                return (t24[:].rearrange("p (r q) -> p q r", q=2)[:, :, :, None]
                        .to_broadcast([T, 2, 12, 3]))

            P0 = new([T, 72]); tt(P0[:], bc72(lo), nawhh[:], AL.max)
            P1 = new([T, 72]); tt(P1[:], bc72(hi), awhh[:], AL.min)
            D = new([T, 72]); tt(D[:], P1[:], P0[:], AL.subtract)
            M0 = new([T, 72]); ts(M0[:], D[:], 0.0, AL.max)
            inter = new([T, 36]); tt(inter[:], M0[:, 0:36], M0[:, 36:72], AL.mult)
            dT = new([T, 24]); tt(dT[:], hi[:], lo[:], AL.subtract)
            dv = dT[:].rearrange("p (r q) -> p r q", q=2)
            areat = new([T, 12]); tt(areat[:], dv[:, :, 0:1], dv[:, :, 1:2], AL.mult)
            un1 = new([T, 36])
            tt(un1[:], areat[:, :, None].to_broadcast([T, 12, 3]), areaa[:], AL.add)
            union = new([T, 36]); tt(union[:], un1[:], inter[:], AL.subtract)
            runi = new([T, 36]); nc.vector.reciprocal(out=runi[:], in_=union[:])
            iou = new([T, 36]); tt(iou[:], inter[:], runi[:], AL.mult)

            # ---------- overlap / argmax / cell / gather offsets ----------
            overlap = new([T, 12])
            nc.vector.reduce_max(out=overlap[:],
                                 in_=iou[:].rearrange("p (r a) -> p r a", a=3),
                                 axis=AX)
            iv = iou[:].rearrange("p (r a) -> p r a", a=3)
            eqB = new([T, 24])
            tt(eqB[:], iv[:, :, 0:2],
               overlap[:, :, None].to_broadcast([T, 12, 2]), AL.is_equal)
            ev = eqB[:].rearrange("p (r e) -> p r e", e=2)
            t2 = new([T, 12]); ts(t2[:], ev[:, :, 1:2], 0.0, AL.is_equal, 1.0, AL.add)
            neq0 = new([T, 12]); ts(neq0[:], ev[:, :, 0:1], 0.0, AL.is_equal)
            anc = new([T, 12]); tt(anc[:], neq0[:], t2[:], AL.mult)

            ca = new([T, 12]); tt(ca[:], anc[:], C(_H_HW, 12), AL.mult)
            cb = new([T, 12]); tt(cb[:], cy4, C(_H_W, 12), AL.mult)
            cc = new([T, 12]); tt(cc[:], ca[:], cb[:], AL.add)
            cell = new([T, 12]); tt(cell[:], cc[:], cx4, AL.add)
            idf = new([T, 12])
            stt(idf[:], cell[:], 85.0, C(_H_BGOFF, 12), AL.mult, AL.add)
            idxi = new([T, 12], I32)
            nc.vector.tensor_copy(out=idxi[:], in_=idf[:])

            # indices to [100,6]: partition p = bh*50+j, col q = s*2+bl
            idx2 = new([100, 6], I32)
            iview = idxi[:].rearrange("p (s bh bl) -> p s bh bl", bh=2, bl=2)
            nc.scalar.dma_start(out=idx2[0:50, :], in_=iview[:, :, 0, :])
            nc.sync.dma_start(out=idx2[50:100, :], in_=iview[:, :, 1, :])

            # ---------- dedup key -> winner (overlaps the gathers) ----------
            # key = m ? cell : 8112 -- within-row compare needs no row offset,
            # so keys fit int16 (cells < 8192)
            om = new([T, 12]); ts(om[:], overlap[:], 0.5, AL.is_gt)
            m = new([T, 12])
            tt(m[:], om[:].rearrange("p (s b) -> p s b", b=4),
               v4[:, None, :].to_broadcast([T, 3, 4]), AL.mult)
            kk = new([T, 12])
            stt(kk[:], cell[:], -SENT, m[:], AL.add, AL.mult)
            key = new([T, 12]); ts(key[:], kk[:], SENT, AL.add)
            key16 = new([T, 12], mybir.dt.int16)
            nc.vector.tensor_copy(out=key16[:], in_=key[:])

            keyT_p = pp.tile([R, T], F32, name="keyT_p")
            nc.tensor.matmul(out=keyT_p[:], lhsT=key[:], rhs=C(_H_EYE, T),
                             start=True, stop=True)
            keyT = new([R, T], mybir.dt.int16)
            nc.vector.tensor_copy(out=keyT[:], in_=keyT_p[:])
            kd2 = nc.dram_tensor("kd2", [R * T], mybir.dt.int16)
            nc.sync.dma_start(out=kd2[:].rearrange("(r k) -> r k", k=T),
                              in_=keyT[:])
            keyB = new([T, R * T], mybir.dt.int16)
            nc.sync.dma_start(out=keyB[:],
                              in_=kd2[:].unsqueeze(0).to_broadcast([T, R * T]))

            # rsqrt of t_wh (dep: t42)
            t42v = t42[:].rearrange("p (s bl c) -> p s bl c", bl=2, c=4)
            rwh2 = new([100, 12])
            nc.vector.reciprocal(out=rwh2[:], in_=t42v[:, :, :, 2:4])
            rstw = new([100, 12]); nc.scalar.sqrt(out=rstw[:], in_=rwh2[:])

            # ---------- 6 indirect gathers (3 pair tiles) ----------
            gpair = [new([100, 8]) for _ in range(3)]
            for q in range(6):
                s_, bl = q // 2, q % 2
                nc.gpsimd.indirect_dma_start(
                    out=gpair[s_][:, bl * 4:(bl + 1) * 4], out_offset=None,
                    in_=outcat_d[:].unsqueeze(1),
                    in_offset=bass.IndirectOffsetOnAxis(ap=idx2[:, q:q + 1],
                                                        axis=0),
                )

            TS2 = new([100, 6])
            winner = new([T, 12])
            winner2 = new([100, 12])   # cols 0:6 winner, cols 6:12 winner*TS

            # dedup tail first on DVE (keyB lands before gather pair 0)
            E = new([T, R * T], mybir.dt.int16)
            tt(E[:], key16[:, :, None].to_broadcast([T, 12, T]), keyB[:],
               AL.is_equal)
            EL = new([T, R * T], mybir.dt.int16)
            tt(EL[:], E[:], lat[:], AL.mult)
            ov = new([T, 12], mybir.dt.int16)
            nc.vector.reduce_max(out=ov[:],
                                 in_=EL[:].rearrange("p (r k) -> p r k", k=T),
                                 axis=AX)
            nov = new([T, 12]); ts(nov[:], ov[:], 0.0, AL.is_equal)
            last_dedup = tt(winner[:], m[:], nov[:], AL.mult)
            wv = winner[:].rearrange("p (s bh bl) -> p s bh bl", bh=2, bl=2)
            nc.scalar.dma_start(out=winner2[0:50, 0:6], in_=wv[:, :, 0, :])
            nc.sync.dma_start(out=winner2[50:100, 0:6], in_=wv[:, :, 1, :])

            def stripe_chain(s_, order_after=None):
                g8 = gpair[s_]
                gv = g8[:].rearrange("p (bl c) -> p bl c", c=4)
                t8 = t42v[:, s_, :, :]
                rcpw = new([100, 4])
                ri = nc.vector.reciprocal(out=rcpw[:], in_=gv[:, :, 2:4])
                if order_after is not None:
                    from concourse.tile import add_dep_helper
                    add_dep_helper(order_after.ins, ri.ins, False,
                                   "keep dedup ahead of stripe chains")
                rspw = new([100, 4]); nc.scalar.sqrt(out=rspw[:], in_=rcpw[:])
                sel = new([100, 8])
                selv = sel[:].rearrange("p (bl c) -> p bl c", c=4)
                tt(selv[:, :, 0:2], gv[:, :, 0:2], t8[:, :, 0:2], AL.subtract)
                tt(selv[:, :, 2:4], rspw[:], rstw[:, 4 * s_:4 * s_ + 4],
                   AL.subtract)
                sq = new([100, 8]); tt(sq[:], sel[:], sel[:], AL.mult)
                nc.vector.reduce_sum(
                    out=TS2[:, 2 * s_:2 * s_ + 2],
                    in_=sq[:].rearrange("p (bl c) -> p bl c", c=4), axis=AX)

            stripe_chain(0)
            stripe_chain(1, order_after=last_dedup)
            stripe_chain(2)

            # ---------- partition-local final reduction ----------
            tt(winner2[:, 6:12], TS2[:], winner2[:, 0:6], AL.mult)
            M1_p = pp.tile([2, 12], F32, name="M1_p")
            nc.tensor.matmul(out=M1_p[:], lhsT=onesU, rhs=winner2[:],
                             start=True, stop=True)
            mx2 = new([2, 6])
            ts(mx2[:], M1_p[:, 0:6], 1.0, AL.max, 2.0, AL.mult)
            rden2 = new([2, 6]); nc.vector.reciprocal(out=rden2[:], in_=mx2[:])
            rl2 = new([2, 6]); tt(rl2[:], M1_p[:, 6:12], rden2[:], AL.mult)
            pt2 = new([2, 1])
            nc.vector.reduce_sum(out=pt2[:], in_=rl2[:], axis=AX)
            tot_p = pp.tile([1, 1], F32, name="tot_p")
            nc.tensor.matmul(out=tot_p[:], lhsT=ones2, rhs=pt2[:],
                             start=True, stop=True)
            p32 = new([1, 1])
            ts(p32[:], tot_p[:], 1.0 / B_TOTAL, AL.mult)

            if use_collective:
                ccin = dp.tile([1, 1], F32, name="ccin")
                ccout = dp.tile([1, 1], F32, name="ccout")
                nc.sync.dma_start(out=ccin[:], in_=p32[:])
                nc.gpsimd.collective_compute(
                    "AllReduce", AL.add,
                    replica_groups=[list(range(NCORES))],
                    ins=[ccin[:].opt()], outs=[ccout[:].opt()],
                )
                nc.sync.dma_start(out=loss_d[:, :], in_=ccout[:])
            else:
                nc.sync.dma_start(out=loss_d[:, :], in_=p32[:])

    nc.compile()
    return nc


_HOST_CONSTS = _host_consts()


def make_in_maps(output0, anchors0, output1, anchors1, output2, anchors2,
                 targets):
    outs = [np.asarray(output0), np.asarray(output1), np.asarray(output2)]
    ancs = [np.asarray(anchors0), np.asarray(anchors1), np.asarray(anchors2)]
    tg = np.asarray(targets)

    aw = np.concatenate([np.tile(a[:, 0], PB) for a in ancs])  # [36] (r, a)
    ah = np.concatenate([np.tile(a[:, 1], PB) for a in ancs])
    awh = np.broadcast_to(np.concatenate([aw, ah]).astype(np.float32), (T, 72))

    in_maps = []
    for c in range(NCORES):
        sl = slice(c * PB, (c + 1) * PB)
        tg16 = tg[sl, :, 1:5].transpose(1, 0, 2).reshape(T, 16).astype(np.float32)
        hostpack = np.ascontiguousarray(
            np.concatenate([tg16, awh, _HOST_CONSTS], axis=1))
        outcat = np.concatenate([o[sl].ravel() for o in outs]).astype(np.float32)
        in_maps.append({"hostpack": hostpack, "outcat": outcat})
    return in_maps


_NC_CACHE = {}


def kernel(output0, anchors0, output1, anchors1, output2, anchors2, targets):
    from concourse.bass_utils import run_bass_kernel_spmd

    if "nc" not in _NC_CACHE:
        _NC_CACHE["nc"] = build_nc(use_collective=False)
    nc = _NC_CACHE["nc"]
    in_maps = make_in_maps(output0, anchors0, output1, anchors1, output2,
                           anchors2, targets)
    res = run_bass_kernel_spmd(nc, in_maps, list(range(NCORES)))
    total = np.float32(0.0)
    for c in range(NCORES):
        total += np.float32(res.results[c]["loss"].reshape(()))
    return np.float32(total)
